# revision 20
# baseline (speedup 1.0000x reference)
"""Trainium2 Bass kernel for the ExemplarBaseline retrieval-kNN model.

Math (per batch b, fully independent across b):
    f      = data.reshape(B*T, CHW) @ W_fe + b_fe            (feature extract)
    d2     = ||f_s - f_t||^2 ; dist = d2**0.25
    sims   = exp(-c * dist)
    numers = 1e-8 + sum_{s<t} sims[s,t] * teach[s, cls]
    score  = numers**gamma / sum_cls ; score[t=0] = 1e-8

Sharding: data-parallel over the batch dim B (128) across 8 NeuronCores,
16 sequences per core.  Host pre-casts x/W to fp8 and pre-transposes x so
the device only does matmuls + a fused epilogue:

  - feats^T [D, tok] = W^T @ x^T via fp8 DoubleRow MMs (24 K-tiles);
    W streams on the scalar HWDGE ring, x on the sync ring (parallel).
  - feats are kept fp8 so the per-sequence Gram AND the sq column-sums
    also run as DoubleRow MMs (4 each instead of 8 plain MMs).
  - per-chunk (4 sequences): one [128, 512] PSUM bank accumulates the 4
    Gram blocks + ONE 128-contraction rank-2 MM each
    ([sqh;sql;1;1] x [1;1;sqh;sql] rows at partitions 0/32/64/96),
    giving psum = G - 0.5*(sq_s + sq_t) = -0.5*d2.
  - batched epilogue on the whole bank: d2 = max(-2*psum, 1e-12);
    dist = exp(0.25*ln d2); sims = exp(-c*dist) (bf16); ONE 3-dim
    affine_select masks s<t per 128-col block; 4 numers MMs; pow via
    batched ln/exp on [128, 40]; normalize; row t=0 := 1e-8; one y DMA.
  - epilogue MM groups are interleaved between the next chunk's feats
    dt-chains so the in-order PE queue never waits on ACT/DVE/GpSimd.
All transcendentals use only Ln/Exp (one ACT table set, no reloads).
"""

import numpy as np
import ml_dtypes

B, T, NC = 128, 128, 10
CHW, D = 3072, 1024
NCORES = 8
BL = B // NCORES          # 16 sequences per core
TOK = BL * T              # 2048 tokens per core
KT = CHW // 128           # 24 contraction tiles
DT = D // 128             # 8 feature tiles
NCHUNK = 4                # token chunks per core
CH = TOK // NCHUNK        # 512 tokens per chunk
BPC = BL // NCHUNK        # 4 sequences per chunk

EPS_NUMER = 1e-8
EPS_D2 = 1e-12

# fp8 Gram/psq via DoubleRow: halves the epilogue MM count but fp8-out
# DVE ops run at half rate (feats evac 740ns vs 376ns bf16), taxing the
# DVE hard.  Measured slower overall than bf16 feats; keep off.
FP8_GRAM = False

_NC_CACHE = {}
LAST_RESULTS = None       # BassKernelResults of the most recent run (for test.py)


def _build_bass():
    import concourse.mybir as mybir
    import concourse.tile as tile
    from concourse import bacc

    f32 = mybir.dt.float32
    bf16 = mybir.dt.bfloat16
    fp8 = mybir.dt.float8e4
    GDT = fp8 if FP8_GRAM else bf16
    AF = mybir.ActivationFunctionType
    OP = mybir.AluOpType
    PM = mybir.MatmulPerfMode

    # The ACT table-set chooser picks the FIRST set containing each function:
    # Exp -> set 0, Ln -> set 5, which makes every Ln<->Exp transition reload
    # tables (~1.3us each).  Both live together in natural_log_exp_and_others;
    # hide them from every other set so the chooser lands there once.
    if not getattr(bacc, "_ln_exp_tables_patched", False):
        orig_tables = bacc.get_activation_tables

        def _patched_tables(arch):
            out = {}
            for name, funcs in orig_tables(arch).items():
                if name != "natural_log_exp_and_others":
                    funcs = funcs - {AF.Ln, AF.Exp}
                out[name] = funcs
            return out

        bacc.get_activation_tables = _patched_tables
        bacc._ln_exp_tables_patched = True

    nc = bacc.Bacc("TRN2", target_bir_lowering=False)

    xT_h = nc.dram_tensor("xT", [CHW, TOK], fp8, kind="ExternalInput")
    W_h = nc.dram_tensor("Wt", [CHW, D], fp8, kind="ExternalInput")
    bfe_h = nc.dram_tensor("bfe", [D], f32, kind="ExternalInput")
    teach_h = nc.dram_tensor("teach", [BL, T, NC], bf16, kind="ExternalInput")
    teach_r = teach_h.rearrange("b s c -> s b c")          # [128, 16, 10]
    negc_h = nc.dram_tensor("negc", [128, 1], f32, kind="ExternalInput")
    gam_h = nc.dram_tensor("gam", [128, 1], f32, kind="ExternalInput")
    # t-major output so each chunk's scores leave in one DMA
    y_h = nc.dram_tensor("y", [T, BL, NC], f32, kind="ExternalOutput")

    xT_r = xT_h.rearrange("(kt p) n -> p kt n", p=128)     # [128, 24, 2048]
    W_r = W_h.rearrange("(kt p) d -> p kt d", p=128)       # [128, 24, 1024]
    bfe_r = bfe_h.rearrange("(dt p) -> p dt", p=128)       # [128, 8]

    with tile.TileContext(nc) as tc:
        with (
            tc.tile_pool(name="cpool", bufs=1) as cpool,
            tc.tile_pool(name="xpool", bufs=2) as xpool,
            tc.tile_pool(name="f2pool", bufs=2) as f2pool,
            tc.tile_pool(name="wpool", bufs=4) as wpool,
            tc.tile_pool(name="spool", bufs=3) as spool,
            tc.tile_pool(name="pfpool", bufs=4, space="PSUM") as pfpool,
            tc.tile_pool(name="psqpool", bufs=1, space="PSUM") as psqpool,
            tc.tile_pool(name="pgpool", bufs=2, space="PSUM") as pgpool,
            tc.tile_pool(name="pnpool", bufs=1, space="PSUM") as pnpool,
        ):
            # ---- persistent tiles -------------------------------------
            W_sb = cpool.tile([128, KT, D], fp8, name="W_sb")
            teach_sb = cpool.tile([128, BL, NC], bf16, name="teach_sb")
            bfe_sb = cpool.tile([128, DT], f32, name="bfe_sb")
            negc_sb = cpool.tile([128, 1], f32, name="negc_sb")
            gam_sb = cpool.tile([128, 1], f32, name="gam_sb")
            eps_sb = cpool.tile([128, 1], f32, name="eps_sb")
            # DoubleRow lhsT for the sq column-sums: ones [128, 2, 32].
            # 32 wide because a 1-wide DR LDWEIGHTS fails the ISA check;
            # the 31 extra output partitions are garbage but cost nothing
            # (matmul time is column-count driven).
            ones2_sb = cpool.tile([128, 2, 32], GDT, name="ones2_sb")
            # rank-2 update operands: pg += sum_p rkA[p,s]*rkB[p,t]
            # Rows live at partitions 0/32/64/96 (engine writes must start
            # 32-aligned), everything else zero:
            #   rkA = [sqh@0; sql@32; 1@64; 1@96],  rkB = [1@0; 1@32;
            #   sqh@64; sql@96] where sqh+sql is the bf16 hi/lo split of
            # -0.5*sq (exact to ~2^-16), so ONE 128-contraction MM (cost =
            # free size only) replaces 4 rank-1 MMs.
            rkA = cpool.tile([128, TOK], bf16, name="rkA")
            rkB = cpool.tile([128, TOK], bf16, name="rkB")
            score_sb = cpool.tile([128, BL, NC], f32, name="score_sb")
            # feats kept fp8, dt-pairs adjacent so Gram/psq can DoubleRow
            fT = cpool.tile([128, DT, TOK], GDT, name="fT")

            # ---- startup DMAs: W on the scalar HWDGE ring, x + misc on
            # the sync ring, so both streams run in parallel.  Growing
            # k-groups pace the first dt-chain's k-inner accumulation.
            # 4 W groups (the HWDGE ring only takes ~4 outstanding issues;
            # a 5th waits for the 1st to finish) on the scalar ring; x on
            # the sync ring in parallel.
            xc0 = xpool.tile([128, KT, CH], fp8, name="xc")
            for k0, k1 in ((0, 2), (2, 8), (8, 16), (16, 24)):
                nc.scalar.dma_start(out=W_sb[:, k0:k1, :], in_=W_r[:, k0:k1, :])
                nc.sync.dma_start(out=xc0[:, k0:k1, :], in_=xT_r[:, k0:k1, 0:CH])
            nc.sync.dma_start(out=teach_sb, in_=teach_r)
            nc.sync.dma_start(out=bfe_sb, in_=bfe_r)
            nc.sync.dma_start(out=negc_sb, in_=negc_h[:, :])
            nc.sync.dma_start(out=gam_sb, in_=gam_h[:, :])

            nc.vector.memset(ones2_sb, 1.0)
            nc.vector.memset(eps_sb, EPS_NUMER)
            nc.vector.memset(rkA, 0.0)
            nc.vector.memset(rkB, 0.0)
            nc.vector.memset(rkA[64:65, :], 1.0)
            nc.vector.memset(rkA[96:97, :], 1.0)
            nc.vector.memset(rkB[0:1, :], 1.0)
            nc.vector.memset(rkB[32:33, :], 1.0)

            def _evac(c, dt_i, pf, f2c):
                # evacuate psum -> fT with per-partition bias add.
                # On DVE (not ACT) so the scalar engine only ever runs
                # Ln/Exp — keeps it on one ACT table set (no reloads).
                csl = slice(c * CH, (c + 1) * CH)
                fsl = fT[:, dt_i, csl]
                nc.vector.tensor_scalar(
                    fsl, pf, bfe_sb[:, dt_i:dt_i + 1], None, op0=OP.add,
                )
                # squares for sq = sum_d f^2 (summed over d via PE)
                nc.vector.tensor_mul(f2c[:, dt_i, :], fsl, fsl)

            def emit_feats_half(c, xc, f2c, half):
                """fp8 DoubleRow feats MMs for 4 of chunk c's 8 dt tiles."""
                for dt_i in range(4 * half, 4 * half + 4):
                    dsl = slice(dt_i * 128, (dt_i + 1) * 128)
                    pf = pfpool.tile([128, CH], f32, name="pf")
                    for k in range(0, KT, 2):
                        nc.tensor.matmul(
                            pf, W_sb[:, k:k + 2, dsl], xc[:, k:k + 2, :],
                            start=(k == 0), stop=(k == KT - 2),
                            perf_mode=PM.DoubleRow,
                        )
                    _evac(c, dt_i, pf, f2c)

            def emit_feats0_wave(xc, f2c):
                """Chunk 0, dt0-3 in k-group WAVES across 4 PSUM banks, so
                the PE consumes each W k-group for all 4 dt tiles as it
                arrives off HBM instead of idling k-serial on dt0 alone."""
                pfs = [pfpool.tile([128, CH], f32, name="pf")
                       for _ in range(4)]
                for k in range(0, KT, 2):
                    for j in range(4):
                        dsl = slice(j * 128, (j + 1) * 128)
                        nc.tensor.matmul(
                            pfs[j], W_sb[:, k:k + 2, dsl], xc[:, k:k + 2, :],
                            start=(k == 0), stop=(k == KT - 2),
                            perf_mode=PM.DoubleRow,
                        )
                for j in range(4):
                    _evac(0, j, pfs[j], f2c)

            def emit_psq(c, f2c):
                """sq for chunk c via DoubleRow PE column-sums of f2;
                feeds the rkA/rkB rank-operand rows on DVE."""
                csl = slice(c * CH, (c + 1) * CH)
                psq = psqpool.tile([32, CH], f32, name="psq")
                if FP8_GRAM:
                    for i in range(0, DT, 2):
                        nc.tensor.matmul(
                            psq, ones2_sb, f2c[:, i:i + 2, :],
                            start=(i == 0), stop=(i == DT - 2),
                            perf_mode=PM.DoubleRow,
                        )
                else:
                    for i in range(DT):
                        nc.tensor.matmul(
                            psq, ones2_sb[:, 0, :], f2c[:, i, :],
                            start=(i == 0), stop=(i == DT - 1),
                        )
                sqf = wpool.tile([1, CH], f32, name="sqf")
                nc.vector.tensor_scalar(sqf, psq[0:1, :], -0.5, None,
                                        op0=OP.mult)
                nc.vector.tensor_copy(rkA[0:1, csl], sqf)                 # hi
                nc.vector.tensor_sub(rkA[32:33, csl], sqf, rkA[0:1, csl])  # lo
                nc.vector.tensor_copy(rkB[64:65, csl], rkA[0:1, csl])
                nc.vector.tensor_copy(rkB[96:97, csl], rkA[32:33, csl])

            def emit_gram(c, s0, ns):
                """Gram + rank MMs for ns sequences of chunk c starting at
                local index s0, into one shared PSUM bank region."""
                pg = pgpool.tile([128, ns, T], f32, name="pg")
                for bi in range(ns):
                    b = c * BPC + s0 + bi
                    tsl = slice(b * T, (b + 1) * T)
                    if FP8_GRAM:
                        for i in range(0, DT, 2):
                            nc.tensor.matmul(
                                pg[:, bi, :], fT[:, i:i + 2, tsl],
                                fT[:, i:i + 2, tsl],
                                start=(i == 0), stop=False,
                                perf_mode=PM.DoubleRow,
                            )
                    else:
                        for i in range(DT):
                            nc.tensor.matmul(
                                pg[:, bi, :], fT[:, i, tsl], fT[:, i, tsl],
                                start=(i == 0), stop=False,
                            )
                    nc.tensor.matmul(
                        pg[:, bi, :], rkA[:, tsl], rkB[:, tsl],
                        start=False, stop=True,
                    )
                return pg

            def emit_chain(c, s0, ns, pg):
                """Batched ACT/GpSimd epilogue over the whole pg bank:
                dist = exp(0.25*ln(-2*psum)) = d2**0.25 straight off PSUM;
                sims = exp(-c*dist) in bf16.  Only masked-out s>=t entries
                can go NaN — off-diagonal d2 ~ 2000 > 0."""
                lt = wpool.tile([128, ns, T], f32, name="lt")
                nc.scalar.activation(lt, pg, AF.Ln, scale=-2.0)
                dist = wpool.tile([128, ns, T], f32, name="dist")
                nc.scalar.activation(dist, lt, AF.Exp, scale=0.25)
                sims = wpool.tile([128, ns, T], bf16, name="sims")
                nc.scalar.activation(sims, dist, AF.Exp, scale=negc_sb)
                # zero s >= t per 128-col block (kills diagonal NaNs too).
                # iota = t_local - s - 1 >= 0 keeps sims exactly where
                # s < t; the [0, ns] pattern dim repeats it per block.
                simsM = wpool.tile([128, ns, T], bf16, name="simsM")
                nc.gpsimd.affine_select(
                    out=simsM, in_=sims,
                    compare_op=OP.is_ge, fill=0.0,
                    base=-1, pattern=[[0, ns], [1, T]], channel_multiplier=-1,
                )
                return simsM

            def emit_numers(c, s0, ns, simsM, pn):
                for bi in range(ns):
                    b = c * BPC + s0 + bi
                    nc.tensor.matmul(
                        pn[:, s0 + bi, :], simsM[:, bi, :], teach_sb[:, b, :],
                        start=True, stop=True,
                    )

            def emit_scores(c, pn):
                """tmp = (numers + eps)**gamma via batched ln/exp on
                [128, 40]; normalize per sequence; row t=0 := eps."""
                l2 = spool.tile([128, BPC, NC], f32, name="l2")
                nc.scalar.activation(l2, pn, AF.Ln, bias=eps_sb)
                tmp = spool.tile([128, BPC, NC], f32, name="tmp")
                nc.scalar.activation(tmp, l2, AF.Exp, scale=gam_sb)
                den = spool.tile([128, BPC, 1], f32, name="den")
                nc.vector.tensor_reduce(
                    den, tmp, axis=mybir.AxisListType.X, op=OP.add,
                )
                rden = spool.tile([128, BPC, 1], f32, name="rden")
                nc.vector.reciprocal(rden, den)
                bsl = slice(c * BPC, (c + 1) * BPC)
                for bi in range(BPC):
                    b = c * BPC + bi
                    nc.vector.tensor_scalar(
                        score_sb[:, b, :], tmp[:, bi, :],
                        rden[:, bi, :], None, op0=OP.mult,
                    )
                nc.vector.memset(score_sb[0:1, bsl, :], EPS_NUMER)
                nc.sync.dma_start(out=y_h[:, bsl, :], in_=score_sb[:, bsl, :])

            # Software pipeline: chunk c-1's epilogue is emitted INSIDE
            # chunk c's feats — psq+gram between the dt halves (their ACT/
            # GpSimd chain then overlaps the second half), numers after.
            xc = xc0
            f2cs = {}
            for c in range(NCHUNK):
                f2cs[c] = f2pool.tile([128, DT, CH], GDT, name="f2c")
                if c == 0:
                    emit_feats0_wave(xc, f2cs[0])
                else:
                    emit_feats_half(c, xc, f2cs[c], 0)
                    emit_psq(c - 1, f2cs[c - 1])
                    pg = emit_gram(c - 1, 0, BPC)
                emit_feats_half(c, xc, f2cs[c], 1)
                if c + 1 < NCHUNK:
                    nxc = xpool.tile([128, KT, CH], fp8, name="xc")
                    nsl = slice((c + 1) * CH, (c + 2) * CH)
                    # chunk-1 x goes on the scalar ring BEHIND W so it
                    # never steals stream bandwidth from the W groups
                    # pacing chunk 0.
                    eng = nc.scalar if c == 0 else nc.sync
                    for k0 in range(0, KT, 12):
                        eng.dma_start(
                            out=nxc[:, k0:k0 + 12, :],
                            in_=xT_r[:, k0:k0 + 12, nsl],
                        )
                    xc = nxc
                if c > 0:
                    simsM = emit_chain(c - 1, 0, BPC, pg)
                    pn = pnpool.tile([128, BPC, NC], f32, name="pn")
                    emit_numers(c - 1, 0, BPC, simsM, pn)
                    emit_scores(c - 1, pn)
            # last chunk: four single-sequence Gram groups, each chain
            # emitted right after its group so the final ACT chains hide
            # behind the following groups' MMs.
            lc = NCHUNK - 1
            emit_psq(lc, f2cs[lc])
            pn = pnpool.tile([128, BPC, NC], f32, name="pn")
            sms = []
            for bi in range(BPC):
                pgi = emit_gram(lc, bi, 1)
                sms.append(emit_chain(lc, bi, 1, pgi))
            for bi in range(BPC):
                emit_numers(lc, bi, 1, sms[bi], pn)
            emit_scores(lc, pn)

    nc.compile()
    return nc


def _get_bass():
    if "nc" not in _NC_CACHE:
        _NC_CACHE["nc"] = _build_bass()
    return _NC_CACHE["nc"]


def make_in_maps(data_t, teaching_signal_t, W_fe, b_fe, c, gamma):
    """Host-side prep: cast to the matmul dtype, transpose x, shard 8 ways."""
    import concourse.mybir as mybir
    mmdt = mybir.dt.np(mybir.dt.float8e4)
    x = np.asarray(data_t, np.float32).reshape(B * T, CHW)
    xbf = x.astype(mmdt)
    Wbf = np.asarray(W_fe, np.float32).astype(mmdt)
    bfe = np.ascontiguousarray(np.asarray(b_fe, np.float32).reshape(D))
    teach = np.asarray(teaching_signal_t, np.float32).astype(ml_dtypes.bfloat16)
    cval = np.float32(np.asarray(c, np.float32).reshape(-1)[0])
    gval = np.float32(np.asarray(gamma, np.float32).reshape(-1)[0])
    negc = np.full((128, 1), -cval, np.float32)
    gam = np.full((128, 1), gval, np.float32)

    in_maps = []
    for core in range(NCORES):
        rows = slice(core * TOK, (core + 1) * TOK)
        xT_c = np.ascontiguousarray(xbf[rows].T)          # [3072, 2048]
        tc_ = np.ascontiguousarray(teach[core * BL:(core + 1) * BL])
        m = dict(
            xT=xT_c, Wt=Wbf, bfe=bfe, teach=tc_,
            negc=negc, gam=gam,
        )
        in_maps.append(m)
    return in_maps


def kernel(responses_t, data_t, teaching_signal_t, W_fe, b_fe, c, gamma):
    global LAST_RESULTS
    from concourse.bass_utils import run_bass_kernel_spmd

    in_maps = make_in_maps(data_t, teaching_signal_t, W_fe, b_fe, c, gamma)
    nc = _get_bass()
    res = run_bass_kernel_spmd(nc, in_maps, core_ids=list(range(NCORES)))
    LAST_RESULTS = res
    # y arrives t-major [T, BL, NC] per core
    y = np.concatenate(
        [r["y"].transpose(1, 0, 2) for r in res.results], axis=0)
    return np.ascontiguousarray(y[:, :, None, :].astype(np.float32))


# revision 21
# speedup vs baseline: 1.0514x; 1.0514x over previous
"""Trainium2 Bass kernel for the ExemplarBaseline retrieval-kNN model.

Math (per batch b, fully independent across b):
    f      = data.reshape(B*T, CHW) @ W_fe + b_fe            (feature extract)
    d2     = ||f_s - f_t||^2 ; dist = d2**0.25
    sims   = exp(-c * dist)
    numers = 1e-8 + sum_{s<t} sims[s,t] * teach[s, cls]
    score  = numers**gamma / sum_cls ; score[t=0] = 1e-8

Sharding: data-parallel over the batch dim B (128) across 8 NeuronCores,
16 sequences per core.  Host pre-casts x/W to fp8 and pre-transposes x so
the device only does matmuls + a fused epilogue:

  - feats^T [D, tok] = W^T @ x^T via fp8 DoubleRow MMs (24 K-tiles);
    W streams on the scalar HWDGE ring, x on the sync ring (parallel).
  - feats are kept fp8 so the per-sequence Gram AND the sq column-sums
    also run as DoubleRow MMs (4 each instead of 8 plain MMs).
  - per-chunk (4 sequences): one [128, 512] PSUM bank accumulates the 4
    Gram blocks + ONE 128-contraction rank-2 MM each
    ([sqh;sql;1;1] x [1;1;sqh;sql] rows at partitions 0/32/64/96),
    giving psum = G - 0.5*(sq_s + sq_t) = -0.5*d2.
  - batched epilogue on the whole bank: d2 = max(-2*psum, 1e-12);
    dist = exp(0.25*ln d2); sims = exp(-c*dist) (bf16); ONE 3-dim
    affine_select masks s<t per 128-col block; 4 numers MMs; pow via
    batched ln/exp on [128, 40]; normalize; row t=0 := 1e-8; one y DMA.
  - epilogue MM groups are interleaved between the next chunk's feats
    dt-chains so the in-order PE queue never waits on ACT/DVE/GpSimd.
All transcendentals use only Ln/Exp (one ACT table set, no reloads).
"""

import numpy as np
import ml_dtypes

B, T, NC = 128, 128, 10
CHW, D = 3072, 1024
NCORES = 8
BL = B // NCORES          # 16 sequences per core
TOK = BL * T              # 2048 tokens per core
KT = CHW // 128           # 24 contraction tiles
DT = D // 128             # 8 feature tiles
NCHUNK = 4                # token chunks per core
CH = TOK // NCHUNK        # 512 tokens per chunk
BPC = BL // NCHUNK        # 4 sequences per chunk

EPS_NUMER = 1e-8
EPS_D2 = 1e-12

# fp8 Gram/psq via DoubleRow: halves the epilogue MM count.  fp8-out DVE
# ops run at half rate (feats evac 740ns vs 376ns bf16) but the PE is the
# bottleneck, so this still measures faster (132.3us vs 135.3us bf16).
FP8_GRAM = True

_NC_CACHE = {}
LAST_RESULTS = None       # BassKernelResults of the most recent run (for test.py)


def _build_bass():
    import concourse.mybir as mybir
    import concourse.tile as tile
    from concourse import bacc

    f32 = mybir.dt.float32
    bf16 = mybir.dt.bfloat16
    fp8 = mybir.dt.float8e4
    GDT = fp8 if FP8_GRAM else bf16
    AF = mybir.ActivationFunctionType
    OP = mybir.AluOpType
    PM = mybir.MatmulPerfMode

    # The ACT table-set chooser picks the FIRST set containing each function:
    # Exp -> set 0, Ln -> set 5, which makes every Ln<->Exp transition reload
    # tables (~1.3us each).  Both live together in natural_log_exp_and_others;
    # hide them from every other set so the chooser lands there once.
    if not getattr(bacc, "_ln_exp_tables_patched", False):
        orig_tables = bacc.get_activation_tables

        def _patched_tables(arch):
            out = {}
            for name, funcs in orig_tables(arch).items():
                if name != "natural_log_exp_and_others":
                    funcs = funcs - {AF.Ln, AF.Exp}
                out[name] = funcs
            return out

        bacc.get_activation_tables = _patched_tables
        bacc._ln_exp_tables_patched = True

    nc = bacc.Bacc("TRN2", target_bir_lowering=False)

    xT_h = nc.dram_tensor("xT", [CHW, TOK], fp8, kind="ExternalInput")
    W_h = nc.dram_tensor("Wt", [CHW, D], fp8, kind="ExternalInput")
    bfe_h = nc.dram_tensor("bfe", [D], f32, kind="ExternalInput")
    teach_h = nc.dram_tensor("teach", [BL, T, NC], bf16, kind="ExternalInput")
    teach_r = teach_h.rearrange("b s c -> s b c")          # [128, 16, 10]
    negc_h = nc.dram_tensor("negc", [128, 1], f32, kind="ExternalInput")
    gam_h = nc.dram_tensor("gam", [128, 1], f32, kind="ExternalInput")
    # t-major output so each chunk's scores leave in one DMA
    y_h = nc.dram_tensor("y", [T, BL, NC], f32, kind="ExternalOutput")

    xT_r = xT_h.rearrange("(kt p) n -> p kt n", p=128)     # [128, 24, 2048]
    W_r = W_h.rearrange("(kt p) d -> p kt d", p=128)       # [128, 24, 1024]
    bfe_r = bfe_h.rearrange("(dt p) -> p dt", p=128)       # [128, 8]

    with tile.TileContext(nc) as tc:
        with (
            tc.tile_pool(name="cpool", bufs=1) as cpool,
            tc.tile_pool(name="xpool", bufs=2) as xpool,
            tc.tile_pool(name="f2pool", bufs=2) as f2pool,
            tc.tile_pool(name="wpool", bufs=4) as wpool,
            tc.tile_pool(name="spool", bufs=3) as spool,
            tc.tile_pool(name="pfpool", bufs=4, space="PSUM") as pfpool,
            tc.tile_pool(name="psqpool", bufs=1, space="PSUM") as psqpool,
            tc.tile_pool(name="pgpool", bufs=2, space="PSUM") as pgpool,
            tc.tile_pool(name="pnpool", bufs=1, space="PSUM") as pnpool,
        ):
            # ---- persistent tiles -------------------------------------
            W_sb = cpool.tile([128, KT, D], fp8, name="W_sb")
            teach_sb = cpool.tile([128, BL, NC], bf16, name="teach_sb")
            bfe_sb = cpool.tile([128, DT], f32, name="bfe_sb")
            negc_sb = cpool.tile([128, 1], f32, name="negc_sb")
            gam_sb = cpool.tile([128, 1], f32, name="gam_sb")
            eps_sb = cpool.tile([128, 1], f32, name="eps_sb")
            # DoubleRow lhsT for the sq column-sums: ones [128, 2, 32].
            # 32 wide because a 1-wide DR LDWEIGHTS fails the ISA check;
            # the 31 extra output partitions are garbage but cost nothing
            # (matmul time is column-count driven).
            ones2_sb = cpool.tile([128, 2, 32], GDT, name="ones2_sb")
            # rank-2 update operands: pg += sum_p rkA[p,s]*rkB[p,t]
            # Rows live at partitions 0/32/64/96 (engine writes must start
            # 32-aligned), everything else zero:
            #   rkA = [sqh@0; sql@32; 1@64; 1@96],  rkB = [1@0; 1@32;
            #   sqh@64; sql@96] where sqh+sql is the bf16 hi/lo split of
            # -0.5*sq (exact to ~2^-16), so ONE 128-contraction MM (cost =
            # free size only) replaces 4 rank-1 MMs.
            rkA = cpool.tile([128, TOK], bf16, name="rkA")
            rkB = cpool.tile([128, TOK], bf16, name="rkB")
            score_sb = cpool.tile([128, BL, NC], f32, name="score_sb")
            # feats kept fp8, dt-pairs adjacent so Gram/psq can DoubleRow
            fT = cpool.tile([128, DT, TOK], GDT, name="fT")

            # ---- startup DMAs: W on the scalar HWDGE ring, x + misc on
            # the sync ring, so both streams run in parallel.  Growing
            # k-groups pace the first dt-chain's k-inner accumulation.
            # 4 W groups (the HWDGE ring only takes ~4 outstanding issues;
            # a 5th waits for the 1st to finish) on the scalar ring; x on
            # the sync ring in parallel.
            xc0 = xpool.tile([128, KT, CH], fp8, name="xc")
            for k0, k1 in ((0, 2), (2, 8), (8, 16), (16, 24)):
                nc.scalar.dma_start(out=W_sb[:, k0:k1, :], in_=W_r[:, k0:k1, :])
                nc.sync.dma_start(out=xc0[:, k0:k1, :], in_=xT_r[:, k0:k1, 0:CH])
            nc.sync.dma_start(out=teach_sb, in_=teach_r)
            nc.sync.dma_start(out=bfe_sb, in_=bfe_r)
            nc.sync.dma_start(out=negc_sb, in_=negc_h[:, :])
            nc.sync.dma_start(out=gam_sb, in_=gam_h[:, :])

            nc.vector.memset(ones2_sb, 1.0)
            nc.vector.memset(eps_sb, EPS_NUMER)
            nc.vector.memset(rkA, 0.0)
            nc.vector.memset(rkB, 0.0)
            nc.vector.memset(rkA[64:65, :], 1.0)
            nc.vector.memset(rkA[96:97, :], 1.0)
            nc.vector.memset(rkB[0:1, :], 1.0)
            nc.vector.memset(rkB[32:33, :], 1.0)

            def _evac(c, dt_i, pf, f2c):
                # evacuate psum -> fT with per-partition bias add.
                # On DVE (not ACT) so the scalar engine only ever runs
                # Ln/Exp — keeps it on one ACT table set (no reloads).
                csl = slice(c * CH, (c + 1) * CH)
                fsl = fT[:, dt_i, csl]
                nc.vector.tensor_scalar(
                    fsl, pf, bfe_sb[:, dt_i:dt_i + 1], None, op0=OP.add,
                )
                # squares for sq = sum_d f^2 (summed over d via PE)
                nc.vector.tensor_mul(f2c[:, dt_i, :], fsl, fsl)

            def emit_feats_half(c, xc, f2c, half):
                """fp8 DoubleRow feats MMs for 4 of chunk c's 8 dt tiles."""
                for dt_i in range(4 * half, 4 * half + 4):
                    dsl = slice(dt_i * 128, (dt_i + 1) * 128)
                    pf = pfpool.tile([128, CH], f32, name="pf")
                    for k in range(0, KT, 2):
                        nc.tensor.matmul(
                            pf, W_sb[:, k:k + 2, dsl], xc[:, k:k + 2, :],
                            start=(k == 0), stop=(k == KT - 2),
                            perf_mode=PM.DoubleRow,
                        )
                    _evac(c, dt_i, pf, f2c)

            def emit_feats0_wave(xc, f2c):
                """Chunk 0, dt0-3 in k-group WAVES across 4 PSUM banks, so
                the PE consumes each W k-group for all 4 dt tiles as it
                arrives off HBM instead of idling k-serial on dt0 alone."""
                pfs = [pfpool.tile([128, CH], f32, name="pf")
                       for _ in range(4)]
                for k in range(0, KT, 2):
                    for j in range(4):
                        dsl = slice(j * 128, (j + 1) * 128)
                        nc.tensor.matmul(
                            pfs[j], W_sb[:, k:k + 2, dsl], xc[:, k:k + 2, :],
                            start=(k == 0), stop=(k == KT - 2),
                            perf_mode=PM.DoubleRow,
                        )
                for j in range(4):
                    _evac(0, j, pfs[j], f2c)

            def emit_psq(c, f2c):
                """sq for chunk c via DoubleRow PE column-sums of f2;
                feeds the rkA/rkB rank-operand rows on DVE."""
                csl = slice(c * CH, (c + 1) * CH)
                psq = psqpool.tile([32, CH], f32, name="psq")
                if FP8_GRAM:
                    for i in range(0, DT, 2):
                        nc.tensor.matmul(
                            psq, ones2_sb, f2c[:, i:i + 2, :],
                            start=(i == 0), stop=(i == DT - 2),
                            perf_mode=PM.DoubleRow,
                        )
                else:
                    for i in range(DT):
                        nc.tensor.matmul(
                            psq, ones2_sb[:, 0, :], f2c[:, i, :],
                            start=(i == 0), stop=(i == DT - 1),
                        )
                sqf = wpool.tile([1, CH], f32, name="sqf")
                nc.vector.tensor_scalar(sqf, psq[0:1, :], -0.5, None,
                                        op0=OP.mult)
                nc.vector.tensor_copy(rkA[0:1, csl], sqf)                 # hi
                nc.vector.tensor_sub(rkA[32:33, csl], sqf, rkA[0:1, csl])  # lo
                nc.vector.tensor_copy(rkB[64:65, csl], rkA[0:1, csl])
                nc.vector.tensor_copy(rkB[96:97, csl], rkA[32:33, csl])

            def emit_gram(c, s0, ns):
                """Gram + rank MMs for ns sequences of chunk c starting at
                local index s0, into one shared PSUM bank region."""
                pg = pgpool.tile([128, ns, T], f32, name="pg")
                for bi in range(ns):
                    b = c * BPC + s0 + bi
                    tsl = slice(b * T, (b + 1) * T)
                    if FP8_GRAM:
                        for i in range(0, DT, 2):
                            nc.tensor.matmul(
                                pg[:, bi, :], fT[:, i:i + 2, tsl],
                                fT[:, i:i + 2, tsl],
                                start=(i == 0), stop=False,
                                perf_mode=PM.DoubleRow,
                            )
                    else:
                        for i in range(DT):
                            nc.tensor.matmul(
                                pg[:, bi, :], fT[:, i, tsl], fT[:, i, tsl],
                                start=(i == 0), stop=False,
                            )
                    nc.tensor.matmul(
                        pg[:, bi, :], rkA[:, tsl], rkB[:, tsl],
                        start=False, stop=True,
                    )
                return pg

            def emit_chain(c, s0, ns, pg):
                """Batched ACT/GpSimd epilogue over the whole pg bank:
                dist = exp(0.25*ln(-2*psum)) = d2**0.25 straight off PSUM;
                sims = exp(-c*dist) in bf16.  Only masked-out s>=t entries
                can go NaN — off-diagonal d2 ~ 2000 > 0."""
                lt = wpool.tile([128, ns, T], f32, name="lt")
                nc.scalar.activation(lt, pg, AF.Ln, scale=-2.0)
                dist = wpool.tile([128, ns, T], f32, name="dist")
                nc.scalar.activation(dist, lt, AF.Exp, scale=0.25)
                sims = wpool.tile([128, ns, T], bf16, name="sims")
                nc.scalar.activation(sims, dist, AF.Exp, scale=negc_sb)
                # zero s >= t per 128-col block (kills diagonal NaNs too).
                # iota = t_local - s - 1 >= 0 keeps sims exactly where
                # s < t; the [0, ns] pattern dim repeats it per block.
                simsM = wpool.tile([128, ns, T], bf16, name="simsM")
                nc.gpsimd.affine_select(
                    out=simsM, in_=sims,
                    compare_op=OP.is_ge, fill=0.0,
                    base=-1, pattern=[[0, ns], [1, T]], channel_multiplier=-1,
                )
                return simsM

            def emit_numers(c, s0, ns, simsM, pn):
                for bi in range(ns):
                    b = c * BPC + s0 + bi
                    nc.tensor.matmul(
                        pn[:, s0 + bi, :], simsM[:, bi, :], teach_sb[:, b, :],
                        start=True, stop=True,
                    )

            def emit_scores(c, pn):
                """tmp = (numers + eps)**gamma via batched ln/exp on
                [128, 40]; normalize per sequence; row t=0 := eps."""
                l2 = spool.tile([128, BPC, NC], f32, name="l2")
                nc.scalar.activation(l2, pn, AF.Ln, bias=eps_sb)
                tmp = spool.tile([128, BPC, NC], f32, name="tmp")
                nc.scalar.activation(tmp, l2, AF.Exp, scale=gam_sb)
                den = spool.tile([128, BPC, 1], f32, name="den")
                nc.vector.tensor_reduce(
                    den, tmp, axis=mybir.AxisListType.X, op=OP.add,
                )
                rden = spool.tile([128, BPC, 1], f32, name="rden")
                nc.vector.reciprocal(rden, den)
                bsl = slice(c * BPC, (c + 1) * BPC)
                for bi in range(BPC):
                    b = c * BPC + bi
                    nc.vector.tensor_scalar(
                        score_sb[:, b, :], tmp[:, bi, :],
                        rden[:, bi, :], None, op0=OP.mult,
                    )
                nc.vector.memset(score_sb[0:1, bsl, :], EPS_NUMER)
                nc.sync.dma_start(out=y_h[:, bsl, :], in_=score_sb[:, bsl, :])

            # Software pipeline: chunk c-1's epilogue is emitted INSIDE
            # chunk c's feats — psq+gram between the dt halves (their ACT/
            # GpSimd chain then overlaps the second half), numers after.
            xc = xc0
            f2cs = {}
            for c in range(NCHUNK):
                f2cs[c] = f2pool.tile([128, DT, CH], GDT, name="f2c")
                if c == 0:
                    emit_feats0_wave(xc, f2cs[0])
                else:
                    emit_feats_half(c, xc, f2cs[c], 0)
                    emit_psq(c - 1, f2cs[c - 1])
                    pg = emit_gram(c - 1, 0, BPC)
                emit_feats_half(c, xc, f2cs[c], 1)
                if c + 1 < NCHUNK:
                    nxc = xpool.tile([128, KT, CH], fp8, name="xc")
                    nsl = slice((c + 1) * CH, (c + 2) * CH)
                    # chunk-1 x goes on the scalar ring BEHIND W so it
                    # never steals stream bandwidth from the W groups
                    # pacing chunk 0.
                    eng = nc.scalar if c == 0 else nc.sync
                    for k0 in range(0, KT, 12):
                        eng.dma_start(
                            out=nxc[:, k0:k0 + 12, :],
                            in_=xT_r[:, k0:k0 + 12, nsl],
                        )
                    xc = nxc
                if c > 0:
                    simsM = emit_chain(c - 1, 0, BPC, pg)
                    pn = pnpool.tile([128, BPC, NC], f32, name="pn")
                    emit_numers(c - 1, 0, BPC, simsM, pn)
                    emit_scores(c - 1, pn)
            # last chunk: four single-sequence Gram groups, each chain
            # emitted right after its group so the final ACT chains hide
            # behind the following groups' MMs.
            lc = NCHUNK - 1
            emit_psq(lc, f2cs[lc])
            pn = pnpool.tile([128, BPC, NC], f32, name="pn")
            sms = []
            for bi in range(BPC):
                pgi = emit_gram(lc, bi, 1)
                sms.append(emit_chain(lc, bi, 1, pgi))
            for bi in range(BPC):
                emit_numers(lc, bi, 1, sms[bi], pn)
            emit_scores(lc, pn)

    nc.compile()
    return nc


def _get_bass():
    if "nc" not in _NC_CACHE:
        _NC_CACHE["nc"] = _build_bass()
    return _NC_CACHE["nc"]


def make_in_maps(data_t, teaching_signal_t, W_fe, b_fe, c, gamma):
    """Host-side prep: cast to the matmul dtype, transpose x, shard 8 ways."""
    import concourse.mybir as mybir
    mmdt = mybir.dt.np(mybir.dt.float8e4)
    x = np.asarray(data_t, np.float32).reshape(B * T, CHW)
    xbf = x.astype(mmdt)
    Wbf = np.asarray(W_fe, np.float32).astype(mmdt)
    bfe = np.ascontiguousarray(np.asarray(b_fe, np.float32).reshape(D))
    teach = np.asarray(teaching_signal_t, np.float32).astype(ml_dtypes.bfloat16)
    cval = np.float32(np.asarray(c, np.float32).reshape(-1)[0])
    gval = np.float32(np.asarray(gamma, np.float32).reshape(-1)[0])
    negc = np.full((128, 1), -cval, np.float32)
    gam = np.full((128, 1), gval, np.float32)

    in_maps = []
    for core in range(NCORES):
        rows = slice(core * TOK, (core + 1) * TOK)
        xT_c = np.ascontiguousarray(xbf[rows].T)          # [3072, 2048]
        tc_ = np.ascontiguousarray(teach[core * BL:(core + 1) * BL])
        m = dict(
            xT=xT_c, Wt=Wbf, bfe=bfe, teach=tc_,
            negc=negc, gam=gam,
        )
        in_maps.append(m)
    return in_maps


def kernel(responses_t, data_t, teaching_signal_t, W_fe, b_fe, c, gamma):
    global LAST_RESULTS
    from concourse.bass_utils import run_bass_kernel_spmd

    in_maps = make_in_maps(data_t, teaching_signal_t, W_fe, b_fe, c, gamma)
    nc = _get_bass()
    res = run_bass_kernel_spmd(nc, in_maps, core_ids=list(range(NCORES)))
    LAST_RESULTS = res
    # y arrives t-major [T, BL, NC] per core
    y = np.concatenate(
        [r["y"].transpose(1, 0, 2) for r in res.results], axis=0)
    return np.ascontiguousarray(y[:, :, None, :].astype(np.float32))


# revision 30
# speedup vs baseline: 1.0660x; 1.0138x over previous
"""Trainium2 Bass kernel for the ExemplarBaseline retrieval-kNN model.

Math (per batch b, fully independent across b):
    f      = data.reshape(B*T, CHW) @ W_fe + b_fe            (feature extract)
    d2     = ||f_s - f_t||^2 ; dist = d2**0.25
    sims   = exp(-c * dist)
    numers = 1e-8 + sum_{s<t} sims[s,t] * teach[s, cls]
    score  = numers**gamma / sum_cls ; score[t=0] = 1e-8

Sharding: data-parallel over the batch dim B (128) across 8 NeuronCores,
16 sequences per core.  Host pre-casts x/W to fp8 and pre-transposes x so
the device only does matmuls + a fused epilogue:

  - feats^T [D, tok] = W^T @ x^T via fp8 DoubleRow MMs (24 K-tiles);
    W streams on the scalar HWDGE ring, x on the sync ring (parallel).
  - feats are kept fp8 so the per-sequence Gram AND the sq column-sums
    also run as DoubleRow MMs (4 each instead of 8 plain MMs).
  - per-chunk (4 sequences): one [128, 512] PSUM bank accumulates the 4
    Gram blocks + ONE 128-contraction rank-2 MM each
    ([sqh;sql;1;1] x [1;1;sqh;sql] rows at partitions 0/32/64/96),
    giving psum = G - 0.5*(sq_s + sq_t) = -0.5*d2.
  - batched epilogue on the whole bank: d2 = max(-2*psum, 1e-12);
    dist = exp(0.25*ln d2); sims = exp(-c*dist) (bf16); ONE 3-dim
    affine_select masks s<t per 128-col block; 4 numers MMs; pow via
    batched ln/exp on [128, 40]; normalize; row t=0 := 1e-8; one y DMA.
  - epilogue MM groups are interleaved between the next chunk's feats
    dt-chains so the in-order PE queue never waits on ACT/DVE/GpSimd.
All transcendentals use only Ln/Exp (one ACT table set, no reloads).
"""

import numpy as np
import ml_dtypes

B, T, NC = 128, 128, 10
CHW, D = 3072, 1024
NCORES = 8
BL = B // NCORES          # 16 sequences per core
TOK = BL * T              # 2048 tokens per core
KT = CHW // 128           # 24 contraction tiles
DT = D // 128             # 8 feature tiles
NCHUNK = 4                # token chunks per core
CH = TOK // NCHUNK        # 512 tokens per chunk
BPC = BL // NCHUNK        # 4 sequences per chunk

EPS_NUMER = 1e-8
EPS_D2 = 1e-12

# fp8 Gram/psq via DoubleRow: halves the epilogue MM count.  fp8-out DVE
# ops run at half rate (feats evac 740ns vs 376ns bf16) but the PE is the
# bottleneck, so this still measures faster (132.3us vs 135.3us bf16).
FP8_GRAM = True

_NC_CACHE = {}
LAST_RESULTS = None       # BassKernelResults of the most recent run (for test.py)


def _build_bass():
    import concourse.mybir as mybir
    import concourse.tile as tile
    from concourse import bacc

    f32 = mybir.dt.float32
    bf16 = mybir.dt.bfloat16
    fp8 = mybir.dt.float8e4
    GDT = fp8 if FP8_GRAM else bf16
    AF = mybir.ActivationFunctionType
    OP = mybir.AluOpType
    PM = mybir.MatmulPerfMode

    # The ACT table-set chooser picks the FIRST set containing each function:
    # Exp -> set 0, Ln -> set 5, which makes every Ln<->Exp transition reload
    # tables (~1.3us each).  Both live together in natural_log_exp_and_others;
    # hide them from every other set so the chooser lands there once.
    if not getattr(bacc, "_ln_exp_tables_patched", False):
        orig_tables = bacc.get_activation_tables

        def _patched_tables(arch):
            out = {}
            for name, funcs in orig_tables(arch).items():
                if name != "natural_log_exp_and_others":
                    funcs = funcs - {AF.Ln, AF.Exp}
                out[name] = funcs
            return out

        bacc.get_activation_tables = _patched_tables
        bacc._ln_exp_tables_patched = True

    nc = bacc.Bacc("TRN2", target_bir_lowering=False)

    xT_h = nc.dram_tensor("xT", [CHW, TOK], fp8, kind="ExternalInput")
    W_h = nc.dram_tensor("Wt", [CHW, D], fp8, kind="ExternalInput")
    bfe_h = nc.dram_tensor("bfe", [D], f32, kind="ExternalInput")
    teach_h = nc.dram_tensor("teach", [BL, T, NC], bf16, kind="ExternalInput")
    teach_r = teach_h.rearrange("b s c -> s b c")          # [128, 16, 10]
    negc_h = nc.dram_tensor("negc", [128, 1], f32, kind="ExternalInput")
    gam_h = nc.dram_tensor("gam", [128, 1], f32, kind="ExternalInput")
    # t-major output so each chunk's scores leave in one DMA
    y_h = nc.dram_tensor("y", [T, BL, NC], f32, kind="ExternalOutput")

    xT_r = xT_h.rearrange("(kt p) n -> p kt n", p=128)     # [128, 24, 2048]
    W_r = W_h.rearrange("(kt p) d -> p kt d", p=128)       # [128, 24, 1024]
    bfe_r = bfe_h.rearrange("(dt p) -> p dt", p=128)       # [128, 8]

    with tile.TileContext(nc) as tc:
        with (
            tc.tile_pool(name="cpool", bufs=1) as cpool,
            tc.tile_pool(name="xpool", bufs=2) as xpool,
            tc.tile_pool(name="f2pool", bufs=2) as f2pool,
            tc.tile_pool(name="wpool", bufs=5) as wpool,
            tc.tile_pool(name="spool", bufs=3) as spool,
            tc.tile_pool(name="pfpool", bufs=4, space="PSUM") as pfpool,
            tc.tile_pool(name="psqpool", bufs=1, space="PSUM") as psqpool,
            tc.tile_pool(name="pgpool", bufs=2, space="PSUM") as pgpool,
            tc.tile_pool(name="pnpool", bufs=1, space="PSUM") as pnpool,
        ):
            # ---- persistent tiles -------------------------------------
            W_sb = cpool.tile([128, KT, D], fp8, name="W_sb")
            teach_sb = cpool.tile([128, BL, NC], bf16, name="teach_sb")
            bfe_sb = cpool.tile([128, DT], f32, name="bfe_sb")
            negc_sb = cpool.tile([128, 1], f32, name="negc_sb")
            gam_sb = cpool.tile([128, 1], f32, name="gam_sb")
            eps_sb = cpool.tile([128, 1], f32, name="eps_sb")
            # DoubleRow lhsT for the sq column-sums: ones [128, 2, 32].
            # 32 wide because a 1-wide DR LDWEIGHTS fails the ISA check;
            # the 31 extra output partitions are garbage but cost nothing
            # (matmul time is column-count driven).
            ones2_sb = cpool.tile([128, 2, 32], GDT, name="ones2_sb")
            # rank-2 update operands: pg += sum_p rkA[p,s]*rkB[p,t]
            # Rows live at partitions 0/32/64/96 (engine writes must start
            # 32-aligned), everything else zero:
            #   rkA = [sqh@0; sql@32; 1@64; 1@96],  rkB = [1@0; 1@32;
            #   sqh@64; sql@96] where sqh+sql is the bf16 hi/lo split of
            # -0.5*sq (exact to ~2^-16), so ONE 128-contraction MM (cost =
            # free size only) replaces 4 rank-1 MMs.
            rkA = cpool.tile([128, TOK], bf16, name="rkA")
            rkB = cpool.tile([128, TOK], bf16, name="rkB")
            score_sb = cpool.tile([128, BL, NC], f32, name="score_sb")
            # feats kept fp8, dt-pairs adjacent so Gram/psq can DoubleRow
            fT = cpool.tile([128, DT, TOK], GDT, name="fT")

            # ---- startup DMAs: W on the scalar HWDGE ring, x + misc on
            # the sync ring, so both streams run in parallel.  Growing
            # k-groups pace the first dt-chain's k-inner accumulation.
            # Startup stream: W + chunk-0 x (4.5 MB) gate the whole first
            # chunk, so split W across BOTH HWDGE rings (~4 outstanding
            # issues max per ring): scalar ring carries W k 0-16, sync
            # ring carries x plus the W tail, keeping both rings busy
            # until the critical set lands.
            xc0 = xpool.tile([128, KT, CH], fp8, name="xc")
            for k0, k1 in ((0, 2), (2, 8), (8, 16)):
                nc.scalar.dma_start(out=W_sb[:, k0:k1, :], in_=W_r[:, k0:k1, :])
            for k0, k1 in ((0, 2), (2, 8), (8, 16), (16, 24)):
                nc.sync.dma_start(out=xc0[:, k0:k1, :], in_=xT_r[:, k0:k1, 0:CH])
            nc.sync.dma_start(out=W_sb[:, 16:24, :], in_=W_r[:, 16:24, :])
            nc.sync.dma_start(out=teach_sb, in_=teach_r)
            nc.sync.dma_start(out=bfe_sb, in_=bfe_r)
            nc.sync.dma_start(out=negc_sb, in_=negc_h[:, :])
            nc.sync.dma_start(out=gam_sb, in_=gam_h[:, :])

            nc.vector.memset(ones2_sb, 1.0)
            nc.vector.memset(eps_sb, EPS_NUMER)
            nc.vector.memset(rkA, 0.0)
            nc.vector.memset(rkB, 0.0)
            nc.vector.memset(rkA[64:65, :], 1.0)
            nc.vector.memset(rkA[96:97, :], 1.0)
            nc.vector.memset(rkB[0:1, :], 1.0)
            nc.vector.memset(rkB[32:33, :], 1.0)

            def _evac(c, dt_i, pf, f2c):
                # evacuate psum -> fT with per-partition bias add.
                # On DVE (not ACT) so the scalar engine only ever runs
                # Ln/Exp — keeps it on one ACT table set (no reloads).
                csl = slice(c * CH, (c + 1) * CH)
                fsl = fT[:, dt_i, csl]
                nc.vector.tensor_scalar(
                    fsl, pf, bfe_sb[:, dt_i:dt_i + 1], None, op0=OP.add,
                )
                # squares for sq = sum_d f^2 (summed over d via PE)
                nc.vector.tensor_mul(f2c[:, dt_i, :], fsl, fsl)

            def emit_feats_half(c, xc, f2c, half):
                """fp8 DoubleRow feats MMs for 4 of chunk c's 8 dt tiles."""
                for dt_i in range(4 * half, 4 * half + 4):
                    dsl = slice(dt_i * 128, (dt_i + 1) * 128)
                    pf = pfpool.tile([128, CH], f32, name="pf")
                    for k in range(0, KT, 2):
                        nc.tensor.matmul(
                            pf, W_sb[:, k:k + 2, dsl], xc[:, k:k + 2, :],
                            start=(k == 0), stop=(k == KT - 2),
                            perf_mode=PM.DoubleRow,
                        )
                    _evac(c, dt_i, pf, f2c)

            def emit_feats0_wave(xc, f2c):
                """Chunk 0, dt0-3 in k-group WAVES across 4 PSUM banks, so
                the PE consumes each W k-group for all 4 dt tiles as it
                arrives off HBM instead of idling k-serial on dt0 alone."""
                pfs = [pfpool.tile([128, CH], f32, name="pf")
                       for _ in range(4)]
                for k in range(0, KT, 2):
                    for j in range(4):
                        dsl = slice(j * 128, (j + 1) * 128)
                        nc.tensor.matmul(
                            pfs[j], W_sb[:, k:k + 2, dsl], xc[:, k:k + 2, :],
                            start=(k == 0), stop=(k == KT - 2),
                            perf_mode=PM.DoubleRow,
                        )
                for j in range(4):
                    _evac(0, j, pfs[j], f2c)

            def emit_psq(c, f2c):
                """sq for chunk c via DoubleRow PE column-sums of f2;
                feeds the rkA/rkB rank-operand rows on DVE."""
                csl = slice(c * CH, (c + 1) * CH)
                psq = psqpool.tile([32, CH], f32, name="psq")
                if FP8_GRAM:
                    for i in range(0, DT, 2):
                        nc.tensor.matmul(
                            psq, ones2_sb, f2c[:, i:i + 2, :],
                            start=(i == 0), stop=(i == DT - 2),
                            perf_mode=PM.DoubleRow,
                        )
                else:
                    for i in range(DT):
                        nc.tensor.matmul(
                            psq, ones2_sb[:, 0, :], f2c[:, i, :],
                            start=(i == 0), stop=(i == DT - 1),
                        )
                sqf = wpool.tile([1, CH], f32, name="sqf")
                nc.vector.tensor_scalar(sqf, psq[0:1, :], -0.5, None,
                                        op0=OP.mult)
                nc.vector.tensor_copy(rkA[0:1, csl], sqf)                 # hi
                nc.vector.tensor_sub(rkA[32:33, csl], sqf, rkA[0:1, csl])  # lo
                # (these duplications were tried on GpSimd to spare the DVE
                # queue — produced corrupt data for all but the last 128
                # columns; keep them on DVE)
                nc.vector.tensor_copy(rkB[64:65, csl], rkA[0:1, csl])
                nc.vector.tensor_copy(rkB[96:97, csl], rkA[32:33, csl])

            def emit_gram_mms_into(c, s, bi, pg):
                """Gram MMs (group left OPEN, no stop) for local sequence
                s of chunk c into region bi of psum tile pg."""
                b = c * BPC + s
                tsl = slice(b * T, (b + 1) * T)
                if FP8_GRAM:
                    for i in range(0, DT, 2):
                        nc.tensor.matmul(
                            pg[:, bi, :], fT[:, i:i + 2, tsl],
                            fT[:, i:i + 2, tsl],
                            start=(i == 0), stop=False,
                            perf_mode=PM.DoubleRow,
                        )
                else:
                    for i in range(DT):
                        nc.tensor.matmul(
                            pg[:, bi, :], fT[:, i, tsl], fT[:, i, tsl],
                            start=(i == 0), stop=False,
                        )

            def emit_rank_one(c, s, bi, pg):
                b = c * BPC + s
                tsl = slice(b * T, (b + 1) * T)
                nc.tensor.matmul(
                    pg[:, bi, :], rkA[:, tsl], rkB[:, tsl],
                    start=False, stop=True,
                )

            def emit_gram(c, s0, ns):
                # NOTE: within one PSUM bank, a sequence's accumulation
                # group (grams ... rank/stop) must fully close before the
                # next sequence's start=True — interleaving open groups in
                # one bank corrupts all but the last-opened one.
                pg = pgpool.tile([128, ns, T], f32, name="pg")
                for bi in range(ns):
                    emit_gram_mms_into(c, s0 + bi, bi, pg)
                    emit_rank_one(c, s0 + bi, bi, pg)
                return pg

            def emit_chain(c, s0, ns, pg):
                """Batched ACT/GpSimd epilogue over the whole pg bank:
                dist = exp(0.25*ln(-2*psum)) = d2**0.25 straight off PSUM;
                sims = exp(-c*dist) in bf16.  Only masked-out s>=t entries
                can go NaN — off-diagonal d2 ~ 2000 > 0."""
                lt = wpool.tile([128, ns, T], f32, name="lt")
                nc.scalar.activation(lt, pg, AF.Ln, scale=-2.0)
                dist = wpool.tile([128, ns, T], f32, name="dist")
                nc.scalar.activation(dist, lt, AF.Exp, scale=0.25)
                sims = wpool.tile([128, ns, T], bf16, name="sims")
                nc.scalar.activation(sims, dist, AF.Exp, scale=negc_sb)
                # zero s >= t per 128-col block (kills diagonal NaNs too).
                # iota = t_local - s - 1 >= 0 keeps sims exactly where
                # s < t; the [0, ns] pattern dim repeats it per block.
                simsM = wpool.tile([128, ns, T], bf16, name="simsM")
                nc.gpsimd.affine_select(
                    out=simsM, in_=sims,
                    compare_op=OP.is_ge, fill=0.0,
                    base=-1, pattern=[[0, ns], [1, T]], channel_multiplier=-1,
                )
                return simsM

            def emit_numers(c, s0, ns, simsM, pn):
                for bi in range(ns):
                    b = c * BPC + s0 + bi
                    nc.tensor.matmul(
                        pn[:, s0 + bi, :], simsM[:, bi, :], teach_sb[:, b, :],
                        start=True, stop=True,
                    )

            def emit_scores(c, pn):
                """tmp = (numers + eps)**gamma via batched ln/exp on
                [128, 40]; normalize per sequence; row t=0 := eps."""
                l2 = spool.tile([128, BPC, NC], f32, name="l2")
                nc.scalar.activation(l2, pn, AF.Ln, bias=eps_sb)
                tmp = spool.tile([128, BPC, NC], f32, name="tmp")
                nc.scalar.activation(tmp, l2, AF.Exp, scale=gam_sb)
                den = spool.tile([128, BPC, 1], f32, name="den")
                nc.vector.tensor_reduce(
                    den, tmp, axis=mybir.AxisListType.X, op=OP.add,
                )
                rden = spool.tile([128, BPC, 1], f32, name="rden")
                nc.vector.reciprocal(rden, den)
                bsl = slice(c * BPC, (c + 1) * BPC)
                for bi in range(BPC):
                    b = c * BPC + bi
                    nc.vector.tensor_scalar(
                        score_sb[:, b, :], tmp[:, bi, :],
                        rden[:, bi, :], None, op0=OP.mult,
                    )
                nc.vector.memset(score_sb[0:1, bsl, :], EPS_NUMER)
                nc.sync.dma_start(out=y_h[:, bsl, :], in_=score_sb[:, bsl, :])

            # Software pipeline: chunk c-1's epilogue is emitted INSIDE
            # chunk c's feats — psq+gram between the dt halves (their ACT/
            # GpSimd chain then overlaps the second half), numers after.
            xc = xc0
            f2cs = {}
            for c in range(NCHUNK):
                f2cs[c] = f2pool.tile([128, DT, CH], GDT, name="f2c")
                if c == 0:
                    emit_feats0_wave(xc, f2cs[0])
                else:
                    emit_feats_half(c, xc, f2cs[c], 0)
                    emit_psq(c - 1, f2cs[c - 1])
                    pg = emit_gram(c - 1, 0, BPC)
                emit_feats_half(c, xc, f2cs[c], 1)
                if c + 1 < NCHUNK:
                    nxc = xpool.tile([128, KT, CH], fp8, name="xc")
                    nsl = slice((c + 1) * CH, (c + 2) * CH)
                    for k0 in range(0, KT, 12):
                        nc.sync.dma_start(
                            out=nxc[:, k0:k0 + 12, :],
                            in_=xT_r[:, k0:k0 + 12, nsl],
                        )
                    xc = nxc
                if c > 0:
                    simsM = emit_chain(c - 1, 0, BPC, pg)
                    if c < NCHUNK - 1:
                        pn = pnpool.tile([128, BPC, NC], f32, name="pn")
                        emit_numers(c - 1, 0, BPC, simsM, pn)
                        emit_scores(c - 1, pn)
            # Tail: chunk-2's numers/scores are deferred to here so they
            # fill the PE while chunk-3's psq DVE chain runs.  Chunk 3
            # goes as four single-sequence groups on two ALTERNATING PSUM
            # banks (two open accumulation groups are fine in different
            # banks): the first two gram groups cover the f2 trail + the
            # rank-operand DVE chain, and each sequence's ACT chain hides
            # behind the next sequence's gram MMs.
            lc = NCHUNK - 1
            pg0 = pgpool.tile([128, 1, T], f32, name="pg")
            emit_gram_mms_into(lc, 0, 0, pg0)
            emit_psq(lc, f2cs[lc])
            pg1 = pgpool.tile([128, 1, T], f32, name="pg")
            emit_gram_mms_into(lc, 1, 0, pg1)
            pn2 = pnpool.tile([128, BPC, NC], f32, name="pn")
            emit_numers(lc - 1, 0, BPC, simsM, pn2)
            emit_scores(lc - 1, pn2)
            emit_rank_one(lc, 0, 0, pg0)
            sm0 = emit_chain(lc, 0, 1, pg0)
            emit_rank_one(lc, 1, 0, pg1)
            sm1 = emit_chain(lc, 1, 1, pg1)
            pg2 = pgpool.tile([128, 1, T], f32, name="pg")
            emit_gram_mms_into(lc, 2, 0, pg2)
            pn3 = pnpool.tile([128, BPC, NC], f32, name="pn")
            emit_numers(lc, 0, 1, sm0, pn3)
            pg3 = pgpool.tile([128, 1, T], f32, name="pg")
            emit_gram_mms_into(lc, 3, 0, pg3)
            emit_rank_one(lc, 2, 0, pg2)
            sm2 = emit_chain(lc, 2, 1, pg2)
            emit_numers(lc, 1, 1, sm1, pn3)
            emit_rank_one(lc, 3, 0, pg3)
            sm3 = emit_chain(lc, 3, 1, pg3)
            emit_numers(lc, 2, 1, sm2, pn3)
            emit_numers(lc, 3, 1, sm3, pn3)
            emit_scores(lc, pn3)

    nc.compile()
    return nc


def _get_bass():
    if "nc" not in _NC_CACHE:
        _NC_CACHE["nc"] = _build_bass()
    return _NC_CACHE["nc"]


def make_in_maps(data_t, teaching_signal_t, W_fe, b_fe, c, gamma):
    """Host-side prep: cast to the matmul dtype, transpose x, shard 8 ways."""
    import concourse.mybir as mybir
    mmdt = mybir.dt.np(mybir.dt.float8e4)
    x = np.asarray(data_t, np.float32).reshape(B * T, CHW)
    xbf = x.astype(mmdt)
    Wbf = np.asarray(W_fe, np.float32).astype(mmdt)
    bfe = np.ascontiguousarray(np.asarray(b_fe, np.float32).reshape(D))
    teach = np.asarray(teaching_signal_t, np.float32).astype(ml_dtypes.bfloat16)
    cval = np.float32(np.asarray(c, np.float32).reshape(-1)[0])
    gval = np.float32(np.asarray(gamma, np.float32).reshape(-1)[0])
    negc = np.full((128, 1), -cval, np.float32)
    gam = np.full((128, 1), gval, np.float32)

    in_maps = []
    for core in range(NCORES):
        rows = slice(core * TOK, (core + 1) * TOK)
        xT_c = np.ascontiguousarray(xbf[rows].T)          # [3072, 2048]
        tc_ = np.ascontiguousarray(teach[core * BL:(core + 1) * BL])
        m = dict(
            xT=xT_c, Wt=Wbf, bfe=bfe, teach=tc_,
            negc=negc, gam=gam,
        )
        in_maps.append(m)
    return in_maps


def kernel(responses_t, data_t, teaching_signal_t, W_fe, b_fe, c, gamma):
    global LAST_RESULTS
    from concourse.bass_utils import run_bass_kernel_spmd

    in_maps = make_in_maps(data_t, teaching_signal_t, W_fe, b_fe, c, gamma)
    nc = _get_bass()
    res = run_bass_kernel_spmd(nc, in_maps, core_ids=list(range(NCORES)))
    LAST_RESULTS = res
    # y arrives t-major [T, BL, NC] per core
    y = np.concatenate(
        [r["y"].transpose(1, 0, 2) for r in res.results], axis=0)
    return np.ascontiguousarray(y[:, :, None, :].astype(np.float32))
